# revision 1
# baseline (speedup 1.0000x reference)
"""Trainium2 Bass kernel for nn_MPCActor: MLP (256->512->512->32, relu/relu/
sigmoid) followed by 100 SGD steps on u (closed form, since the per-element
recurrence u <- a*u + b with a = 1-2*lr*q, b = -lr*p has the exact solution
u_N = a^N u0 - 0.5*(p/q)*(1 - a^N)).

Data parallel over 8 NeuronCores: batch 32768 -> 4096 rows per core, MLP
weights replicated. Activations are kept feature-on-partition / batch-on-free
so weights serve as the stationary matmul operand in their natural [in, out]
layout; obs tiles are transposed on the PE. Matmuls run in bf16 (fp32
accumulate in PSUM); everything after the sigmoid stays fp32.

Only the 8 W3 columns that the u-update actually reads (q_u = cols 12:16,
p_u = cols 28:32) are computed; x_init never enters the gradient.

Engine split per batch tile: PE transposes + matmuls; PSUM drains alternate
between ACT (relu w/ bias) and DVE (fused add-bias+max0 tensor_scalar);
the f32->bf16 obs cast runs on the otherwise idle GpSimd; layer 2 runs its
K-chunk loop outermost so its matmuls start as soon as the first y1 chunk
is drained.
"""

import numpy as np

import concourse.bass as bass
import concourse.mybir as mybir
import concourse.tile as tile
from concourse import bacc, masks
from concourse.bass_utils import run_bass_kernel_spmd

NCORES = 8
BATCH = 32768
BPC = BATCH // NCORES  # 4096 rows per core
OBS = 256
HID = 512
NQP = 8  # q_u (4) + p_u (4) columns of W3 that matter
BT = 512  # batch tile (matmul moving free dim)
NT = BPC // BT  # 8 batch tiles per core
LR = 0.01
F32 = mybir.dt.float32
MD = mybir.dt.bfloat16  # matmul dtype

_CACHE = {}


def _build_nc():
    nc = bacc.Bacc(
        trn_type="TRN2", target_bir_lowering=False, debug=False, num_devices=NCORES
    )
    obs = nc.declare_dram_parameter("obs", [BPC, OBS], F32, isOutput=False).ap()
    u0 = nc.declare_dram_parameter("u0", [BPC, 4], F32, isOutput=False).ap()
    w1 = nc.declare_dram_parameter("w1", [OBS, HID], F32, isOutput=False).ap()
    w2 = nc.declare_dram_parameter("w2", [HID, HID], F32, isOutput=False).ap()
    w3 = nc.declare_dram_parameter("w3", [HID, NQP], F32, isOutput=False).ap()
    b1 = nc.declare_dram_parameter("b1", [128, 4], F32, isOutput=False).ap()
    b2 = nc.declare_dram_parameter("b2", [128, 4], F32, isOutput=False).ap()
    b3 = nc.declare_dram_parameter("b3", [NQP, 1], F32, isOutput=False).ap()
    uo = nc.declare_dram_parameter("uo", [BPC, 4], F32, isOutput=True).ap()

    AF = mybir.ActivationFunctionType
    ALU = mybir.AluOpType

    with tile.TileContext(nc) as tc:
        from contextlib import ExitStack

        with ExitStack() as ctx:
            singles = ctx.enter_context(tc.tile_pool(name="singles", bufs=1))
            p_obsf = ctx.enter_context(tc.tile_pool(name="obsf", bufs=2))
            p_obsb = ctx.enter_context(tc.tile_pool(name="obsb", bufs=2))
            p_obsT = ctx.enter_context(tc.tile_pool(name="obsT", bufs=2))
            p_y1 = ctx.enter_context(tc.tile_pool(name="y1", bufs=2))
            p_y2 = ctx.enter_context(tc.tile_pool(name="y2", bufs=2))
            p_qp = ctx.enter_context(tc.tile_pool(name="qp", bufs=2))
            p_cf = ctx.enter_context(tc.tile_pool(name="cf", bufs=2))
            # PSUM budget is 8 banks: ot 2 + y1 2 + y2 2 + z3 1 + qpt 1
            pp_ot = ctx.enter_context(tc.tile_pool(name="ppot", bufs=2, space="PSUM"))
            pp_y1 = ctx.enter_context(tc.tile_pool(name="ppy1", bufs=2, space="PSUM"))
            pp_y2 = ctx.enter_context(tc.tile_pool(name="ppy2", bufs=2, space="PSUM"))
            pp_z3 = ctx.enter_context(tc.tile_pool(name="ppz3", bufs=1, space="PSUM"))
            pp_qpt = ctx.enter_context(tc.tile_pool(name="ppqpt", bufs=1, space="PSUM"))

            # ---- one-time: weights (cast to bf16), biases, identities ----
            w1f = singles.tile([128, 2, HID], F32)
            nc.sync.dma_start(out=w1f, in_=w1.rearrange("(kc p) m -> p kc m", p=128))
            w1s = singles.tile([128, 2, HID], MD)
            nc.vector.tensor_copy(out=w1s, in_=w1f)

            w2f = singles.tile([128, 4, HID], F32)
            nc.sync.dma_start(out=w2f, in_=w2.rearrange("(kc p) m -> p kc m", p=128))
            w2s = singles.tile([128, 4, HID], MD)
            nc.vector.tensor_copy(out=w2s, in_=w2f)

            w3f = singles.tile([128, 4, NQP], F32)
            nc.sync.dma_start(out=w3f, in_=w3.rearrange("(kc p) m -> p kc m", p=128))
            w3s = singles.tile([128, 4, NQP], MD)
            nc.vector.tensor_copy(out=w3s, in_=w3f)

            b1s = singles.tile([128, 4], F32)
            nc.sync.dma_start(out=b1s, in_=b1)
            b2s = singles.tile([128, 4], F32)
            nc.sync.dma_start(out=b2s, in_=b2)
            b3s = singles.tile([NQP, 1], F32)
            nc.sync.dma_start(out=b3s, in_=b3)

            ident = singles.tile([128, 128], MD)
            masks.make_identity(nc, ident[:])
            id8 = singles.tile([8, 8], F32)
            masks.make_identity(nc, id8[:])

            obs_t = obs.rearrange("(t c p) f -> t p c f", p=128, c=4)
            u0_t = u0.rearrange("(t c p) j -> p t c j", p=128, c=4)
            uo_t = uo.rearrange("(t c p) j -> p t c j", p=128, c=4)


            def drain(dst, src, bias_ap, m):
                if m % 2 == 0:
                    nc.scalar.activation(
                        out=dst, in_=src, func=AF.Relu, bias=bias_ap, scale=1.0
                    )
                else:
                    nc.vector.tensor_scalar(dst, src, bias_ap, 0.0, ALU.add, ALU.max)

            for it in range(NT):
                # load obs tile [128, 4, 256]; cast on GpSimd
                obsf = p_obsf.tile([128, 4, OBS], F32)
                nc.sync.dma_start(out=obsf, in_=obs_t[it])
                obsb = p_obsb.tile([128, 4, OBS], MD)
                nc.vector.tensor_copy(out=obsb, in_=obsf)

                # transpose to obsT [256, BT] as 2 chunks of [128, BT]
                obsT = []
                for f in range(2):
                    ps = pp_ot.tile([128, BT], MD, tag="ot")
                    for c in range(4):
                        nc.tensor.transpose(
                            ps[:, c * 128 : (c + 1) * 128],
                            obsb[:, c, f * 128 : (f + 1) * 128],
                            ident[:],
                        )
                    ot = p_obsT.tile([128, BT], MD, tag=f"obsT{f}")
                    nc.vector.tensor_copy(out=ot, in_=ps)
                    obsT.append(ot)

                # layer 1: y1T[m] = relu(W1[:, m].T @ obsT + b1[m])
                y1 = []
                for m in range(4):
                    ps = pp_y1.tile([128, BT], F32, tag="y1")
                    for kc in range(2):
                        nc.tensor.matmul(
                            ps,
                            w1s[:, kc, m * 128 : (m + 1) * 128],
                            obsT[kc],
                            start=(kc == 0),
                            stop=(kc == 1),
                        )
                    t = p_y1.tile([128, BT], MD, tag=f"y1_{m}")
                    drain(t, ps, b1s[:, m : m + 1], m)
                    y1.append(t)

                # layer 2
                y2 = []
                for m in range(4):
                    ps = pp_y2.tile([128, BT], F32, name="ps2", tag="y2")
                    for kc in range(4):
                        nc.tensor.matmul(
                            ps,
                            w2s[:, kc, m * 128 : (m + 1) * 128],
                            y1[kc],
                            start=(kc == 0),
                            stop=(kc == 3),
                        )
                    t = p_y2.tile([128, BT], MD, tag=f"y2_{m}")
                    drain(t, ps, b2s[:, m : m + 1], m + 1)
                    y2.append(t)

                # layer 3 (only the 8 useful output columns), sigmoid
                ps3 = pp_z3.tile([NQP, BT], F32, tag="z3")
                for kc in range(4):
                    nc.tensor.matmul(
                        ps3, w3s[:, kc, :], y2[kc], start=(kc == 0), stop=(kc == 3)
                    )
                qpT = p_qp.tile([NQP, BT], F32, tag="qpT")
                nc.scalar.activation(
                    out=qpT, in_=ps3, func=AF.Sigmoid, bias=b3s[:, 0:1], scale=1.0
                )

                # transpose to batch-major [128, 4 chunks, 8]; free the bank fast
                psq = pp_qpt.tile([128, 4, NQP], F32, tag="qpt")
                for c in range(4):
                    nc.tensor.transpose(
                        psq[:, c, :], qpT[:, c * 128 : (c + 1) * 128], id8[:]
                    )
                # closed-form 100-step update on [128, 4, 4] fp32
                q = psq[:, :, 0:4]
                p = psq[:, :, 4:8]
                TS = nc.vector.tensor_scalar

                u0b = p_cf.tile([128, 4, 4], F32, tag="u0b")
                nc.sync.dma_start(out=u0b, in_=u0_t[:, it])

                a = p_cf.tile([128, 4, 4], F32, tag="a")  # a = 1 - 2*lr*q
                nc.scalar.activation(out=a, in_=q, func=AF.Copy, bias=1.0, scale=-2.0 * LR)
                a2 = p_cf.tile([128, 4, 4], F32, tag="a2")
                nc.vector.tensor_mul(a2, a, a)
                a4 = p_cf.tile([128, 4, 4], F32, tag="a4")
                nc.vector.tensor_mul(a4, a2, a2)
                a8 = p_cf.tile([128, 4, 4], F32, tag="a8")
                nc.vector.tensor_mul(a8, a4, a4)
                a16 = p_cf.tile([128, 4, 4], F32, tag="a16")
                nc.vector.tensor_mul(a16, a8, a8)
                a32 = p_cf.tile([128, 4, 4], F32, tag="a32")
                nc.vector.tensor_mul(a32, a16, a16)
                a64 = p_cf.tile([128, 4, 4], F32, tag="a64")
                nc.vector.tensor_mul(a64, a32, a32)
                a96 = p_cf.tile([128, 4, 4], F32, tag="a96")
                nc.vector.tensor_mul(a96, a64, a32)
                A = p_cf.tile([128, 4, 4], F32, tag="A")
                nc.vector.tensor_mul(A, a96, a4)

                n1 = p_cf.tile([128, 4, 4], F32, tag="n1")  # 0.5*(1-A)
                nc.scalar.activation(out=n1, in_=A, func=AF.Copy, bias=0.5, scale=-0.5)
                rq = p_cf.tile([128, 4, 4], F32, tag="rq")
                nc.vector.reciprocal(rq, q)
                r = p_cf.tile([128, 4, 4], F32, tag="r")
                nc.vector.tensor_mul(r, p, rq)
                tt = p_cf.tile([128, 4, 4], F32, tag="tt")
                nc.vector.tensor_mul(tt, r, n1)
                mm = p_cf.tile([128, 4, 4], F32, tag="mm")
                nc.vector.tensor_mul(mm, A, u0b)
                uob = p_cf.tile([128, 4, 4], F32, tag="uob")
                nc.vector.tensor_sub(uob, mm, tt)
                nc.sync.dma_start(out=uo_t[:, it], in_=uob)
    nc.finalize()
    return nc


def _get_nc():
    if "nc" not in _CACHE:
        _CACHE["nc"] = _build_nc()
    return _CACHE["nc"]


def kernel(obs, x_init, u_init, W1, b1, W2, b2, W3, b3):
    obs = np.ascontiguousarray(np.asarray(obs, dtype=np.float32))
    u_init = np.ascontiguousarray(np.asarray(u_init, dtype=np.float32))
    W1 = np.asarray(W1, dtype=np.float32)
    W2 = np.asarray(W2, dtype=np.float32)
    W3 = np.asarray(W3, dtype=np.float32)
    b1 = np.asarray(b1, dtype=np.float32)
    b2 = np.asarray(b2, dtype=np.float32)
    b3 = np.asarray(b3, dtype=np.float32)

    # only columns 12:16 (q_u) and 28:32 (p_u) of the MLP head are used
    w3u = np.ascontiguousarray(np.concatenate([W3[:, 12:16], W3[:, 28:32]], axis=1))
    b3u = np.ascontiguousarray(np.concatenate([b3[12:16], b3[28:32]])[:, None])
    b1p = np.ascontiguousarray(b1.reshape(4, 128).T)  # [128, m] chunks
    b2p = np.ascontiguousarray(b2.reshape(4, 128).T)
    w1c = np.ascontiguousarray(W1)
    w2c = np.ascontiguousarray(W2)

    nc = _get_nc()
    in_maps = []
    for i in range(NCORES):
        in_maps.append(
            {
                "obs": obs[i * BPC : (i + 1) * BPC],
                "u0": u_init[i * BPC : (i + 1) * BPC],
                "w1": w1c,
                "w2": w2c,
                "w3": w3u,
                "b1": b1p,
                "b2": b2p,
                "b3": b3u,
            }
        )
    import os

    kw = {}
    if os.environ.get("BASSK_TRACE"):
        kw = {"trace": True, "tmpdir": os.environ.get("BASSK_TRACE_DIR") or None}
    res = run_bass_kernel_spmd(nc, in_maps, list(range(NCORES)), **kw)
    _CACHE["last_result"] = res
    out = np.concatenate([res.results[i]["uo"] for i in range(NCORES)], axis=0)
    return out.astype(np.float32)



# revision 8
# speedup vs baseline: 1.3761x; 1.3761x over previous
"""Trainium2 Bass kernel for nn_MPCActor: MLP (256->512->512->8-useful-cols,
relu/relu/sigmoid) followed by the closed-form equivalent of 100 SGD steps on
u (u_N = A*u0 - 0.5*(p/q)*(1-A), A = (1-2*lr*q)^100).

Data parallel over 8 NeuronCores: batch 32768 -> 4096 rows per core, weights
replicated. All matmuls run in fp8(e4m3) with perf_mode=DoubleRow (K=256 per
instruction, 2x PE throughput). The obs transpose, weight layouts, fp8 casts
and scale folding are done on the host:
  obsT = obs.T/8 (fp8)      W1h = 8*W1 (fp8)   -> psum1 = z1 exactly
  y1   = relu(z1+b1) (fp8)  W2h = 64*W2 (fp8)  -> psum2 = 64*z2
  y2'  = relu(psum2+64*b2) = 64*y2 (fp8)
  W3h  = 16*W3[:, useful] padded to 16 cols    -> psum3 = 1024*z3
  qpT  = sigmoid(psum3/1024 + b3)  (ACT, feature-major [16,512])
Batch tiles are processed in pairs sharing each stationary weight load so
LDWEIGHTS can hide behind the previous matmul. PSUM drains alternate between
ACT (relu w/ bias) and DVE (tensor_scalar add-bias+max0); the closed-form u
update runs on the otherwise idle GPSIMD (with DVE doing the PSUM reads and
the reciprocal).
"""

import numpy as np
import ml_dtypes

import concourse.bass as bass
import concourse.mybir as mybir
import concourse.tile as tile
from concourse import bacc, masks
from concourse.bass_utils import run_bass_kernel_spmd

NCORES = 8
BATCH = 32768
BPC = BATCH // NCORES  # 4096 rows per core
OBS = 256
HID = 512
NQP = 16  # q_u (4) + p_u (4) + 8 zero-pad cols (step%16==0 for DoubleRow)
BT = 512  # batch tile (matmul moving free dim)
NT = BPC // BT  # 8 batch tiles per core
LR = 0.01
F32 = mybir.dt.float32
FP8 = mybir.dt.float8e4
DR = mybir.MatmulPerfMode.DoubleRow

_CACHE = {}


def _build_nc():
    nc = bacc.Bacc(
        trn_type="TRN2", target_bir_lowering=False, debug=False, num_devices=NCORES
    )
    obsT = nc.declare_dram_parameter("obsT", [NT, 128, 2, BT], FP8, isOutput=False).ap()
    u0 = nc.declare_dram_parameter("u0", [BPC, 4], F32, isOutput=False).ap()
    w1 = nc.declare_dram_parameter("w1", [128, 2, HID], FP8, isOutput=False).ap()
    w2 = nc.declare_dram_parameter("w2", [128, 4, HID], FP8, isOutput=False).ap()
    w3 = nc.declare_dram_parameter("w3", [128, 4, NQP], FP8, isOutput=False).ap()
    b1 = nc.declare_dram_parameter("b1", [128, 4], F32, isOutput=False).ap()
    b2 = nc.declare_dram_parameter("b2", [128, 4], F32, isOutput=False).ap()
    b3 = nc.declare_dram_parameter("b3", [NQP, 1], F32, isOutput=False).ap()
    uo = nc.declare_dram_parameter("uo", [BPC, 4], F32, isOutput=True).ap()

    AF = mybir.ActivationFunctionType
    ALU = mybir.AluOpType

    with tile.TileContext(nc) as tc:
        from contextlib import ExitStack

        with ExitStack() as ctx:
            singles = ctx.enter_context(tc.tile_pool(name="singles", bufs=1))
            p_obs = ctx.enter_context(tc.tile_pool(name="obs", bufs=4))
            p_y1 = ctx.enter_context(tc.tile_pool(name="y1", bufs=2))
            p_y2 = ctx.enter_context(tc.tile_pool(name="y2", bufs=2))
            p_qp = ctx.enter_context(tc.tile_pool(name="qp", bufs=2))
            p_cf = ctx.enter_context(tc.tile_pool(name="cf", bufs=2))
            # PSUM budget is 8 banks: y 4 + z3 2 + psq 2
            pp_y = ctx.enter_context(tc.tile_pool(name="ppy", bufs=4, space="PSUM"))
            pp_z3 = ctx.enter_context(tc.tile_pool(name="ppz3", bufs=2, space="PSUM"))
            pp_q = ctx.enter_context(tc.tile_pool(name="ppq", bufs=2, space="PSUM"))

            # ---- one-time: weights (already fp8/scaled on host), biases ----
            w1s = singles.tile([128, 2, HID], FP8)
            nc.sync.dma_start(out=w1s, in_=w1)
            w2s = singles.tile([128, 4, HID], FP8)
            nc.sync.dma_start(out=w2s, in_=w2)
            w3s = singles.tile([128, 4, NQP], FP8)
            nc.sync.dma_start(out=w3s, in_=w3)
            b1s = singles.tile([128, 4], F32)
            nc.sync.dma_start(out=b1s, in_=b1)
            b2s = singles.tile([128, 4], F32)
            nc.sync.dma_start(out=b2s, in_=b2)
            b3s = singles.tile([NQP, 1], F32)
            nc.sync.dma_start(out=b3s, in_=b3)
            id16 = singles.tile([NQP, NQP], F32)
            masks.make_identity(nc, id16[:])

            u0_t = u0.rearrange("(t c p) j -> p t c j", p=128, c=4)
            uo_t = uo.rearrange("(t c p) j -> p t c j", p=128, c=4)

            def drain(dst, src, bias_ap, on_act):
                if on_act:
                    nc.scalar.activation(
                        out=dst, in_=src, func=AF.Relu, bias=bias_ap, scale=1.0
                    )
                else:
                    nc.vector.tensor_scalar(dst, src, bias_ap, 0.0, ALU.add, ALU.max)

            for g in range(NT // 2):
                ts = (2 * g, 2 * g + 1)
                obsb = {}
                u0b = {}
                for t in ts:
                    ob = p_obs.tile([128, 2, BT], FP8, tag=f"obs{t % 4}")
                    nc.sync.dma_start(out=ob, in_=obsT[t])
                    obsb[t] = ob
                    ub = p_cf.tile([128, 4, 4], F32, tag=f"u0b{t % 2}")
                    nc.sync.dma_start(out=ub, in_=u0_t[:, t])
                    u0b[t] = ub

                # layer 1: psum = z1 (scales folded on host); pair shares LDW
                y1 = {
                    t: p_y1.tile(
                        [128, 4, HID], FP8, name=f"y1_{t % 2}", tag=f"y1_{t % 2}"
                    )
                    for t in ts
                }
                ps1 = {}
                for m in range(4):
                    for t in ts:
                        ps = pp_y.tile([128, BT], F32, name="ps1", tag="y")
                        nc.tensor.matmul(
                            ps,
                            w1s[:, 0:2, m * 128 : (m + 1) * 128],
                            obsb[t],
                            start=True,
                            stop=True,
                            perf_mode=DR,
                        )
                        ps1[(t, m)] = ps
                    for t in ts:
                        drain(
                            y1[t][:, m, :], ps1[(t, m)], b1s[:, m : m + 1], m % 2 == 0
                        )

                # layer 2: psum = 64*z2; drain y2' = relu(psum + 64*b2) = 64*y2
                y2 = {
                    t: p_y2.tile(
                        [128, 4, HID], FP8, name=f"y2_{t % 2}", tag=f"y2_{t % 2}"
                    )
                    for t in ts
                }
                for m in range(4):
                    ps2 = {
                        t: pp_y.tile([128, BT], F32, name="ps2", tag="y") for t in ts
                    }
                    for kc in range(2):
                        for t in ts:
                            nc.tensor.matmul(
                                ps2[t],
                                w2s[:, 2 * kc : 2 * kc + 2, m * 128 : (m + 1) * 128],
                                y1[t][:, 2 * kc : 2 * kc + 2, :],
                                start=(kc == 0),
                                stop=(kc == 1),
                                perf_mode=DR,
                            )
                    for t in ts:
                        drain(y2[t][:, m, :], ps2[t], b2s[:, m : m + 1], m % 2 == 1)

                # layer 3 (16 padded cols): psum = 1024*z3; sigmoid on ACT
                ps3 = {
                    t: pp_z3.tile([NQP, BT], F32, name="ps3", tag="z3") for t in ts
                }
                for kc in range(2):
                    for t in ts:
                        nc.tensor.matmul(
                            ps3[t],
                            w3s[:, 2 * kc : 2 * kc + 2, :],
                            y2[t][:, 2 * kc : 2 * kc + 2, :],
                            start=(kc == 0),
                            stop=(kc == 1),
                            perf_mode=DR,
                        )
                for t in ts:
                    qpT = p_qp.tile([NQP, BT], F32, tag=f"qpT{t % 2}")
                    nc.scalar.activation(
                        out=qpT,
                        in_=ps3[t],
                        func=AF.Sigmoid,
                        bias=b3s[:, 0:1],
                        scale=1.0 / 1024.0,
                    )
                    # transpose to batch-major psq[128, c, 16]
                    psq = pp_q.tile([128, 4, NQP], F32, tag="psq")
                    for c in range(4):
                        nc.tensor.transpose(
                            psq[:, c, :], qpT[:, c * 128 : (c + 1) * 128], id16[:]
                        )
                    q = psq[:, :, 0:4]
                    p = psq[:, :, 4:8]
                    # DVE reads PSUM: a = 1-2*lr*q, rq = 1/q, r = p/q
                    TS = nc.vector.tensor_scalar
                    a = p_cf.tile([128, 4, 4], F32, tag="a")
                    TS(a, q, -2.0 * LR, 1.0, ALU.mult, ALU.add)
                    rq = p_cf.tile([128, 4, 4], F32, tag="rq")
                    nc.vector.reciprocal(rq, q)
                    r = p_cf.tile([128, 4, 4], F32, tag="r")
                    nc.vector.tensor_mul(r, p, rq)
                    # GPSIMD (SBUF only): A = a^100 and the final update
                    GM = nc.gpsimd.tensor_mul
                    a2 = p_cf.tile([128, 4, 4], F32, tag="a2")
                    GM(a2, a, a)
                    a4 = p_cf.tile([128, 4, 4], F32, tag="a4")
                    GM(a4, a2, a2)
                    a8 = p_cf.tile([128, 4, 4], F32, tag="a8")
                    GM(a8, a4, a4)
                    a16 = p_cf.tile([128, 4, 4], F32, tag="a16")
                    GM(a16, a8, a8)
                    a32 = p_cf.tile([128, 4, 4], F32, tag="a32")
                    GM(a32, a16, a16)
                    a64 = p_cf.tile([128, 4, 4], F32, tag="a64")
                    GM(a64, a32, a32)
                    a96 = p_cf.tile([128, 4, 4], F32, tag="a96")
                    GM(a96, a64, a32)
                    A = p_cf.tile([128, 4, 4], F32, tag="A")
                    GM(A, a96, a4)
                    n1 = p_cf.tile([128, 4, 4], F32, tag="n1")  # 0.5*(1-A)
                    nc.gpsimd.tensor_scalar(n1, A, -0.5, 0.5, ALU.mult, ALU.add)
                    tt = p_cf.tile([128, 4, 4], F32, tag="tt")
                    GM(tt, r, n1)
                    mm = p_cf.tile([128, 4, 4], F32, tag="mm")
                    GM(mm, A, u0b[t])
                    uob = p_cf.tile([128, 4, 4], F32, tag="uob")
                    nc.gpsimd.tensor_sub(uob, mm, tt)
                    nc.sync.dma_start(out=uo_t[:, t], in_=uob)
    nc.finalize()
    return nc


def _get_nc():
    if "nc" not in _CACHE:
        _CACHE["nc"] = _build_nc()
    return _CACHE["nc"]


FP8NP = ml_dtypes.float8_e4m3  # TRN float8e4: bias 7, max normal +-240


def _to_fp8(x):
    return np.ascontiguousarray(np.clip(x, -240.0, 240.0)).astype(FP8NP)


def kernel(obs, x_init, u_init, W1, b1, W2, b2, W3, b3):
    obs = np.asarray(obs, dtype=np.float32)
    u_init = np.ascontiguousarray(np.asarray(u_init, dtype=np.float32))
    W1 = np.asarray(W1, dtype=np.float32)
    W2 = np.asarray(W2, dtype=np.float32)
    W3 = np.asarray(W3, dtype=np.float32)
    b1 = np.asarray(b1, dtype=np.float32)
    b2 = np.asarray(b2, dtype=np.float32)
    b3 = np.asarray(b3, dtype=np.float32)

    # weights with fp8 scale folding (see module docstring)
    w1h = _to_fp8((8.0 * W1).reshape(2, 128, HID).transpose(1, 0, 2))
    w2h = _to_fp8((64.0 * W2).reshape(4, 128, HID).transpose(1, 0, 2))
    w3u = np.zeros((HID, NQP), dtype=np.float32)
    w3u[:, 0:4] = 16.0 * W3[:, 12:16]  # q_u
    w3u[:, 4:8] = 16.0 * W3[:, 28:32]  # p_u
    w3h = _to_fp8(w3u.reshape(4, 128, NQP).transpose(1, 0, 2))
    b1p = np.ascontiguousarray(b1.reshape(4, 128).T)
    b2p = np.ascontiguousarray(64.0 * b2.reshape(4, 128).T)
    b3p = np.zeros((NQP, 1), dtype=np.float32)
    b3p[0:4, 0] = b3[12:16]
    b3p[4:8, 0] = b3[28:32]

    nc = _get_nc()
    in_maps = []
    for i in range(NCORES):
        oc = obs[i * BPC : (i + 1) * BPC]  # [4096, 256]
        # [t, p, kc, n] = obs[t*512+n, kc*128+p] / 8
        obsT = _to_fp8(oc.reshape(NT, BT, 2, 128).transpose(0, 3, 2, 1) / 8.0)
        in_maps.append(
            {
                "obsT": obsT,
                "u0": u_init[i * BPC : (i + 1) * BPC],
                "w1": w1h,
                "w2": w2h,
                "w3": w3h,
                "b1": b1p,
                "b2": b2p,
                "b3": b3p,
            }
        )
    import os

    kw = {}
    if os.environ.get("BASSK_TRACE"):
        kw = {"trace": True, "tmpdir": os.environ.get("BASSK_TRACE_DIR") or None}
    res = run_bass_kernel_spmd(nc, in_maps, list(range(NCORES)), **kw)
    _CACHE["last_result"] = res
    out = np.concatenate([res.results[i]["uo"] for i in range(NCORES)], axis=0)
    return out.astype(np.float32)


# revision 15
# speedup vs baseline: 1.5803x; 1.1484x over previous
"""Trainium2 Bass kernel for nn_MPCActor: MLP (256->512->512->8-useful-cols,
relu/relu/sigmoid) followed by the closed-form equivalent of 100 SGD steps on
u (u_N = A*u0 - 0.5*(p/q)*(1-A), A = (1-2*lr*q)^100).

Data parallel over 8 NeuronCores: batch 32768 -> 4096 rows per core, weights
replicated. All matmuls run in fp8(e4m3) with perf_mode=DoubleRow (K=256 per
instruction, 2x PE throughput). The obs transpose, weight layouts, fp8 casts
and scale folding are done on the host:
  obsT = obs.T/8 (fp8)      W1h = 8*W1 (fp8)   -> psum1 = z1 exactly
  y1   = relu(z1+b1) (fp8)  W2h = 64*W2 (fp8)  -> psum2 = 64*z2
  y2'  = relu(psum2+64*b2) = 64*y2 (fp8)
  W3h  = 16*W3[:, useful] padded to 16 cols    -> psum3 = 1024*z3
  qpT  = sigmoid(psum3/1024 + b3)  (ACT, feature-major [16,512])

The drains are the bottleneck (PSUM reads are 1 elem/cycle on ACT/DVE and DMA
has no PSUM route), so PSUM y-tiles span two banks ([128,2,512]) and each
drain moves 128x1024 values in one instruction; three such groups rotate so
both drain engines stay busy while the PE fills the third. Batch tiles are
processed in pairs sharing each stationary weight load; the closed-form u
update is batched per pair on the otherwise idle GPSIMD.
"""

import numpy as np
import ml_dtypes

import concourse.bass as bass
import concourse.mybir as mybir
import concourse.tile as tile
from concourse import bacc, masks
from concourse.bass_utils import run_bass_kernel_spmd

NCORES = 8
BATCH = 32768
BPC = BATCH // NCORES  # 4096 rows per core
OBS = 256
HID = 512
NQP = 16  # q_u (4) + p_u (4) + 8 zero-pad cols (step%16==0 for DoubleRow)
BT = 512  # batch tile (matmul moving free dim)
NT = BPC // BT  # 8 batch tiles per core
LR = 0.01
F32 = mybir.dt.float32
FP8 = mybir.dt.float8e4
DR = mybir.MatmulPerfMode.DoubleRow

_CACHE = {}


def _build_nc(zero_bias):
    nc = bacc.Bacc(
        trn_type="TRN2", target_bir_lowering=False, debug=False, num_devices=NCORES
    )
    obsT = nc.declare_dram_parameter("obsT", [NT, 128, 2, BT], FP8, isOutput=False).ap()
    u0 = nc.declare_dram_parameter("u0", [BPC, 4], F32, isOutput=False).ap()
    w1 = nc.declare_dram_parameter("w1", [128, 2, HID], FP8, isOutput=False).ap()
    w2 = nc.declare_dram_parameter("w2", [128, 4, HID], FP8, isOutput=False).ap()
    w3 = nc.declare_dram_parameter("w3", [128, 4, NQP], FP8, isOutput=False).ap()
    b1 = nc.declare_dram_parameter("b1", [128, 4], F32, isOutput=False).ap()
    b2 = nc.declare_dram_parameter("b2", [128, 4], F32, isOutput=False).ap()
    b3 = nc.declare_dram_parameter("b3", [NQP, 1], F32, isOutput=False).ap()
    uo = nc.declare_dram_parameter("uo", [BPC, 4], F32, isOutput=True).ap()

    AF = mybir.ActivationFunctionType
    ALU = mybir.AluOpType

    with tile.TileContext(nc) as tc:
        from contextlib import ExitStack

        with ExitStack() as ctx:
            singles = ctx.enter_context(tc.tile_pool(name="singles", bufs=1))
            p_obs = ctx.enter_context(tc.tile_pool(name="obs", bufs=4))
            p_y1 = ctx.enter_context(tc.tile_pool(name="y1", bufs=2))
            p_y2 = ctx.enter_context(tc.tile_pool(name="y2", bufs=2))
            p_qp = ctx.enter_context(tc.tile_pool(name="qp", bufs=2))
            p_cf = ctx.enter_context(tc.tile_pool(name="cf", bufs=2))
            # PSUM budget is 8 banks: y 3x2 + z3 1 + psq 1
            pp_y = ctx.enter_context(tc.tile_pool(name="ppy", bufs=3, space="PSUM"))
            pp_z3 = ctx.enter_context(tc.tile_pool(name="ppz3", bufs=1, space="PSUM"))
            pp_q = ctx.enter_context(tc.tile_pool(name="ppq", bufs=1, space="PSUM"))

            # ---- one-time: weights (already fp8/scaled on host), biases ----
            w1s = singles.tile([128, 2, HID], FP8)
            nc.sync.dma_start(out=w1s, in_=w1)
            w2s = singles.tile([128, 4, HID], FP8)
            nc.sync.dma_start(out=w2s, in_=w2)
            w3s = singles.tile([128, 4, NQP], FP8)
            nc.sync.dma_start(out=w3s, in_=w3)
            b1s = singles.tile([128, 4], F32)
            nc.sync.dma_start(out=b1s, in_=b1)
            b2s = singles.tile([128, 4], F32)
            nc.sync.dma_start(out=b2s, in_=b2)
            b3s = singles.tile([NQP, 1], F32)
            nc.sync.dma_start(out=b3s, in_=b3)
            id16 = singles.tile([NQP, NQP], F32)
            masks.make_identity(nc, id16[:])

            u0_t = u0.rearrange("(t c p) j -> p t c j", p=128, c=4)
            uo_t = uo.rearrange("(t c p) j -> p t c j", p=128, c=4)

            def drain2(dst, src, bias_sb, mp, on_act):
                # dst [128, 2, 512] fp8 <- relu(src [128, 2, 512] psum + bias)
                # bias is per (partition, m-chunk); engine bias operands are
                # per-partition only, so nonzero bias needs per-chunk drains.
                if zero_bias:
                    if on_act:
                        nc.scalar.activation(
                            out=dst, in_=src, func=AF.Relu, bias=0.0, scale=1.0
                        )
                    else:
                        nc.vector.tensor_scalar(dst, src, 0.0, None, ALU.max)
                else:
                    for mi in range(2):
                        b = bias_sb[:, 2 * mp + mi : 2 * mp + mi + 1]
                        if on_act:
                            nc.scalar.activation(
                                out=dst[:, mi, :],
                                in_=src[:, mi, :],
                                func=AF.Relu,
                                bias=b,
                                scale=1.0,
                            )
                        else:
                            nc.vector.tensor_scalar(
                                dst[:, mi, :], src[:, mi, :], b, 0.0, ALU.add, ALU.max
                            )

            for g in range(NT // 2):
                ts = (2 * g, 2 * g + 1)
                obsb = {}
                for t in ts:
                    ob = p_obs.tile([128, 2, BT], FP8, name="ob", tag="obs")
                    nc.sync.dma_start(out=ob, in_=obsT[t])
                    obsb[t] = ob
                u0b = p_cf.tile([128, 2, 4, 4], F32, tag="u0b")
                nc.sync.dma_start(out=u0b, in_=u0_t[:, 2 * g : 2 * g + 2])

                # layer 1: psum = z1 (scales folded on host); pair shares LDW;
                # psum groups span 2 banks so one drain moves 128x1024 values
                y1 = {
                    t: p_y1.tile(
                        [128, 4, HID], FP8, name=f"y1_{t % 2}", tag=f"y1_{t % 2}"
                    )
                    for t in ts
                }
                for mp in range(2):
                    ps1 = {
                        t: pp_y.tile([128, 2, BT], F32, name="ps1", tag="y") for t in ts
                    }
                    for mi in range(2):
                        m = 2 * mp + mi
                        for t in ts:
                            nc.tensor.matmul(
                                ps1[t][:, mi, :],
                                w1s[:, 0:2, m * 128 : (m + 1) * 128],
                                obsb[t],
                                start=True,
                                stop=True,
                                perf_mode=DR,
                            )
                    for i, t in enumerate(ts):
                        drain2(
                            y1[t][:, 2 * mp : 2 * mp + 2, :],
                            ps1[t],
                            b1s,
                            mp,
                            (mp + i) % 2 == 0,
                        )

                # layer 2: psum = 64*z2; drain y2' = relu(psum + 64*b2) = 64*y2
                y2 = {
                    t: p_y2.tile(
                        [128, 4, HID], FP8, name=f"y2_{t % 2}", tag=f"y2_{t % 2}"
                    )
                    for t in ts
                }
                for mp in range(2):
                    ps2 = {
                        t: pp_y.tile([128, 2, BT], F32, name="ps2", tag="y") for t in ts
                    }
                    for mi in range(2):
                        m = 2 * mp + mi
                        for kc in range(2):
                            for t in ts:
                                nc.tensor.matmul(
                                    ps2[t][:, mi, :],
                                    w2s[
                                        :, 2 * kc : 2 * kc + 2, m * 128 : (m + 1) * 128
                                    ],
                                    y1[t][:, 2 * kc : 2 * kc + 2, :],
                                    start=(kc == 0),
                                    stop=(kc == 1),
                                    perf_mode=DR,
                                )
                    for i, t in enumerate(ts):
                        drain2(
                            y2[t][:, 2 * mp : 2 * mp + 2, :],
                            ps2[t],
                            b2s,
                            mp,
                            (mp + i) % 2 == 1,
                        )

                # layer 3 (16 padded cols): psum = 1024*z3; fused bias+sigmoid
                # on ACT, then PE transpose to batch-major and a copy to SBUF
                qp = p_qp.tile([128, 2, 4, NQP], F32, tag="qp")
                for i, t in enumerate(ts):
                    ps3 = pp_z3.tile([NQP, BT], F32, name="ps3", tag="z3")
                    for kc in range(2):
                        nc.tensor.matmul(
                            ps3,
                            w3s[:, 2 * kc : 2 * kc + 2, :],
                            y2[t][:, 2 * kc : 2 * kc + 2, :],
                            start=(kc == 0),
                            stop=(kc == 1),
                            perf_mode=DR,
                        )
                    qpT = p_qp.tile([NQP, BT], F32, tag="qpT")
                    nc.scalar.activation(
                        out=qpT,
                        in_=ps3,
                        func=AF.Sigmoid,
                        bias=b3s[:, 0:1],
                        scale=1.0 / 1024.0,
                    )
                    psq = pp_q.tile([128, 4, NQP], F32, tag="psq")
                    for c in range(4):
                        nc.tensor.transpose(
                            psq[:, c, :], qpT[:, c * 128 : (c + 1) * 128], id16[:]
                        )
                    if i == 0:
                        nc.scalar.copy(out=qp[:, i], in_=psq)
                    else:
                        nc.vector.tensor_copy(out=qp[:, i], in_=psq)

                # closed form, batched over the pair: GPSIMD + one DVE recip
                q = qp[:, :, :, 0:4]
                p = qp[:, :, :, 4:8]
                SH = [128, 2, 4, 4]
                a = p_cf.tile(SH, F32, tag="a")
                nc.gpsimd.tensor_scalar(a, q, -2.0 * LR, 1.0, ALU.mult, ALU.add)
                rq = p_cf.tile(SH, F32, tag="rq")
                nc.vector.reciprocal(rq, q)
                GM = nc.gpsimd.tensor_mul
                a2 = p_cf.tile(SH, F32, tag="a2")
                GM(a2, a, a)
                a4 = p_cf.tile(SH, F32, tag="a4")
                GM(a4, a2, a2)
                a8 = p_cf.tile(SH, F32, tag="a8")
                GM(a8, a4, a4)
                a16 = p_cf.tile(SH, F32, tag="a16")
                GM(a16, a8, a8)
                a32 = p_cf.tile(SH, F32, tag="a32")
                GM(a32, a16, a16)
                a64 = p_cf.tile(SH, F32, tag="a64")
                GM(a64, a32, a32)
                a96 = p_cf.tile(SH, F32, tag="a96")
                GM(a96, a64, a32)
                A = p_cf.tile(SH, F32, tag="A")
                GM(A, a96, a4)
                n1 = p_cf.tile(SH, F32, tag="n1")  # 0.5*(1-A)
                nc.gpsimd.tensor_scalar(n1, A, -0.5, 0.5, ALU.mult, ALU.add)
                r = p_cf.tile(SH, F32, tag="r")
                GM(r, p, rq)
                tt = p_cf.tile(SH, F32, tag="tt")
                GM(tt, r, n1)
                mm = p_cf.tile(SH, F32, tag="mm")
                GM(mm, A, u0b)
                uob = p_cf.tile(SH, F32, tag="uob")
                nc.gpsimd.tensor_sub(uob, mm, tt)
                nc.sync.dma_start(out=uo_t[:, 2 * g : 2 * g + 2], in_=uob)
    nc.finalize()
    return nc


def _get_nc(zero_bias):
    key = ("nc", zero_bias)
    if key not in _CACHE:
        _CACHE[key] = _build_nc(zero_bias)
    return _CACHE[key]


FP8NP = ml_dtypes.float8_e4m3  # TRN float8e4: bias 7, max normal +-240


def _to_fp8(x):
    return np.ascontiguousarray(np.clip(x, -240.0, 240.0)).astype(FP8NP)


def kernel(obs, x_init, u_init, W1, b1, W2, b2, W3, b3):
    obs = np.asarray(obs, dtype=np.float32)
    u_init = np.ascontiguousarray(np.asarray(u_init, dtype=np.float32))
    W1 = np.asarray(W1, dtype=np.float32)
    W2 = np.asarray(W2, dtype=np.float32)
    W3 = np.asarray(W3, dtype=np.float32)
    b1 = np.asarray(b1, dtype=np.float32)
    b2 = np.asarray(b2, dtype=np.float32)
    b3 = np.asarray(b3, dtype=np.float32)

    # weights with fp8 scale folding (see module docstring)
    w1h = _to_fp8((8.0 * W1).reshape(2, 128, HID).transpose(1, 0, 2))
    w2h = _to_fp8((64.0 * W2).reshape(4, 128, HID).transpose(1, 0, 2))
    w3u = np.zeros((HID, NQP), dtype=np.float32)
    w3u[:, 0:4] = 16.0 * W3[:, 12:16]  # q_u
    w3u[:, 4:8] = 16.0 * W3[:, 28:32]  # p_u
    w3h = _to_fp8(w3u.reshape(4, 128, NQP).transpose(1, 0, 2))
    b1p = np.ascontiguousarray(b1.reshape(4, 128).T)
    b2p = np.ascontiguousarray(64.0 * b2.reshape(4, 128).T)
    b3p = np.zeros((NQP, 1), dtype=np.float32)
    b3p[0:4, 0] = b3[12:16]
    b3p[4:8, 0] = b3[28:32]

    zero_bias = bool(np.all(b1 == 0.0) and np.all(b2 == 0.0))
    nc = _get_nc(zero_bias)
    in_maps = []
    for i in range(NCORES):
        oc = obs[i * BPC : (i + 1) * BPC]  # [4096, 256]
        # [t, p, kc, n] = obs[t*512+n, kc*128+p] / 8
        obsT = _to_fp8(oc.reshape(NT, BT, 2, 128).transpose(0, 3, 2, 1) / 8.0)
        in_maps.append(
            {
                "obsT": obsT,
                "u0": u_init[i * BPC : (i + 1) * BPC],
                "w1": w1h,
                "w2": w2h,
                "w3": w3h,
                "b1": b1p,
                "b2": b2p,
                "b3": b3p,
            }
        )
    import os

    kw = {}
    if os.environ.get("BASSK_TRACE"):
        kw = {"trace": True, "tmpdir": os.environ.get("BASSK_TRACE_DIR") or None}
    res = run_bass_kernel_spmd(nc, in_maps, list(range(NCORES)), **kw)
    _CACHE["last_result"] = res
    out = np.concatenate([res.results[i]["uo"] for i in range(NCORES)], axis=0)
    return out.astype(np.float32)
